# revision 6
# baseline (speedup 1.0000x reference)
"""AED distillation loss kernel for 8 TRN2 NeuronCores.

Data-parallel: batch dim B=1024 sharded 8-ways (128 rows/core, one SBUF
partition per row). Each core streams its 4x[128,32000] logits once,
producing the mimic shard plus per-row partial statistics; the tiny
O(B) combination (log/sqrt/cos/min/mean) happens on host in float64.

Per chunk [128, W] the device computes (j = model 0..3):
  s1_j  = sum_c exp(o_j)            ScalarE Exp + accum
  s2_j  = sum_c exp(o_j/T)          ScalarE Exp(scale=1/T) + accum
  s3_j  = sum_c exp(teacher_j/T)    ScalarE Exp(scale=0.8/T on u_j) + accum
  A_j   = sum_c e3_j*mimic          VectorE scalar_tensor_tensor + accum
  B_j   = sum_c e3_j*o_j            VectorE stt + accum
  ssq_j = sum_c o_j^2               ScalarE Square + accum
  dot_jk= sum_c o_j*o_k             VectorE stt + accum (6 pairs)
  mimic = 0.25*sum4                 ScalarE Copy(scale=.25) -> DRAM
where u_j = o_j + 0.0625*sum4 so teacher_j/T = (0.8/T)*u_j, and
mimic - o_j recovers from A_j - B_j.

Host closes the books:
  CE_j   = ln(s1_j) - o_j[t]
  KD_j   = (T^2/C) * ((0.2/T)*(A_j-B_j)/s3_j - (ln s3_j - ln s2_j))
  cos_jk = (dot_jk - tv_j tv_k)/sqrt((ssq_j-tv_j^2)(ssq_k-tv_k^2))
  L      = sum_j(CE_j+KD_j) + 0.7*min_j CE_j + 0.3*sum_{j<k} cos_jk
"""

import sys

import numpy as np

if "/opt/trn_rl_repo" not in sys.path:
    sys.path.insert(0, "/opt/trn_rl_repo")

import concourse.bacc as bacc
import concourse.tile as tile
from concourse import mybir
from concourse.bass_utils import run_bass_kernel_spmd

B, C, NCORES = 1024, 32000, 8
BL = B // NCORES          # 128 batch rows per core = SBUF partition count
W = 3200                  # chunk width along C
NCH = C // W              # 10 chunks
T = 3.0

F32 = mybir.dt.float32
F16 = mybir.dt.float16
AF = mybir.ActivationFunctionType
ALU = mybir.AluOpType

# stats column layout (per chunk)
NS_ACT = 16               # s1(4) s2(4) s3(4) ssq(4)   — written by ScalarE
NS_DVE = 14               # A(4) B(4) dots(6)          — written by VectorE
PAIRS = [(0, 1), (0, 2), (0, 3), (1, 2), (1, 3), (2, 3)]

_CACHED = {}


def _build():
    nc = bacc.Bacc(None, target_bir_lowering=False)
    o_ext = [
        nc.declare_dram_parameter(f"o{j}", [BL, C], F32, isOutput=False)
        for j in range(4)
    ]
    mimic_ext = nc.declare_dram_parameter("mimic", [BL, C], F32, isOutput=True)
    sa_ext = nc.declare_dram_parameter("sa", [BL, NCH * NS_ACT], F32, isOutput=True)
    sv_ext = nc.declare_dram_parameter("sv", [BL, NCH * NS_DVE], F32, isOutput=True)

    with tile.TileContext(nc) as tc:
        with (
            tc.tile_pool(name="io", bufs=5) as io_pool,
            tc.tile_pool(name="ob", bufs=4) as ob_pool,
            tc.tile_pool(name="mid", bufs=2) as mid_pool,
            tc.tile_pool(name="mo", bufs=2) as mo_pool,
            tc.tile_pool(name="scr", bufs=2) as scr_pool,
            tc.tile_pool(name="st", bufs=1) as st_pool,
        ):
            sa = st_pool.tile([BL, NCH * NS_ACT], F32, tag="sa")
            sv = st_pool.tile([BL, NCH * NS_DVE], F32, tag="sv")

            def sla(c, k):
                return sa[:, c * NS_ACT + k : c * NS_ACT + k + 1]

            def slv(c, k):
                return sv[:, c * NS_DVE + k : c * NS_DVE + k + 1]

            for c in range(NCH):
                lo, hi = c * W, (c + 1) * W
                o_t = []
                for j in range(4):
                    t_ = io_pool.tile([BL, W], F32, tag="o")
                    nc.sync.dma_start(out=t_, in_=o_ext[j][:, lo:hi])
                    o_t.append(t_)
                ob = []
                for j in range(4):
                    t_ = ob_pool.tile([BL, W], F16, tag="ob")
                    nc.vector.tensor_copy(out=t_, in_=o_t[j])
                    ob.append(t_)
                s01 = mid_pool.tile([BL, W], F16, tag="sps")
                s23 = mid_pool.tile([BL, W], F16, tag="sps")
                sum4 = mid_pool.tile([BL, W], F16, tag="sum4")
                nc.vector.tensor_add(out=s01, in0=ob[0], in1=ob[1])
                nc.vector.tensor_add(out=s23, in0=ob[2], in1=ob[3])
                nc.vector.tensor_add(out=sum4, in0=s01, in1=s23)
                mimicb = mid_pool.tile([BL, W], F16, tag="mimicb")
                nc.vector.tensor_scalar_mul(mimicb, sum4, 0.25)
                mimic_f = mo_pool.tile([BL, W], F32, tag="mimicf")
                nc.scalar.activation(
                    out=mimic_f, in_=sum4, func=AF.Copy, bias=0.0, scale=0.25
                )
                nc.sync.dma_start(out=mimic_ext[:, lo:hi], in_=mimic_f)

                for j in range(4):
                    # CE / log_s normalizers ride ScalarE with free accum
                    e_scr = scr_pool.tile([BL, W], F16, tag="escr")
                    nc.scalar.activation(
                        out=e_scr, in_=o_t[j], func=AF.Exp,
                        bias=0.0, scale=1.0, accum_out=sla(c, 0 + j),
                    )
                    e_scr2 = scr_pool.tile([BL, W], F16, tag="escr")
                    nc.scalar.activation(
                        out=e_scr2, in_=o_t[j], func=AF.Exp,
                        bias=0.0, scale=1.0 / T, accum_out=sla(c, 4 + j),
                    )
                    sq_scr = scr_pool.tile([BL, W], F16, tag="escr")
                    nc.scalar.activation(
                        out=sq_scr, in_=o_t[j], func=AF.Square,
                        bias=0.0, scale=1.0, accum_out=sla(c, 12 + j),
                    )
                    # teacher exp: u_j = o_j + 0.0625*sum4; e3 = exp(0.8/T * u)
                    u_t = scr_pool.tile([BL, W], F16, tag="u")
                    nc.vector.scalar_tensor_tensor(
                        out=u_t, in0=sum4, scalar=0.0625, in1=ob[j],
                        op0=ALU.mult, op1=ALU.add,
                    )
                    e3 = scr_pool.tile([BL, W], F16, tag="e3")
                    nc.scalar.activation(
                        out=e3, in_=u_t, func=AF.Exp,
                        bias=0.0, scale=0.8 / T, accum_out=sla(c, 8 + j),
                    )
                    pa = scr_pool.tile([BL, W], F16, tag="pscr")
                    nc.vector.scalar_tensor_tensor(
                        out=pa, in0=e3, scalar=1.0, in1=mimicb,
                        op0=ALU.mult, op1=ALU.mult, accum_out=slv(c, 0 + j),
                    )
                    pb = scr_pool.tile([BL, W], F16, tag="pscr")
                    nc.vector.scalar_tensor_tensor(
                        out=pb, in0=e3, scalar=1.0, in1=ob[j],
                        op0=ALU.mult, op1=ALU.mult, accum_out=slv(c, 4 + j),
                    )
                for idx, (j, k) in enumerate(PAIRS):
                    pd = scr_pool.tile([BL, W], F16, tag="pscr")
                    nc.vector.scalar_tensor_tensor(
                        out=pd, in0=ob[j], scalar=1.0, in1=ob[k],
                        op0=ALU.mult, op1=ALU.mult, accum_out=slv(c, 8 + idx),
                    )
            nc.sync.dma_start(out=sa_ext[:], in_=sa)
            nc.sync.dma_start(out=sv_ext[:], in_=sv)
    nc.compile()
    return nc


def _get_nc():
    if "nc" not in _CACHED:
        _CACHED["nc"] = _build()
    return _CACHED["nc"]


def kernel(outputs1, outputs2, outputs3, outputs4, targets):
    outs = [
        np.ascontiguousarray(np.asarray(o, dtype=np.float32))
        for o in (outputs1, outputs2, outputs3, outputs4)
    ]
    tgt = np.asarray(targets).astype(np.int64)

    nc = _get_nc()
    in_maps = [
        {f"o{j}": outs[j][i * BL : (i + 1) * BL] for j in range(4)}
        for i in range(NCORES)
    ]
    res = run_bass_kernel_spmd(nc, in_maps, core_ids=list(range(NCORES)))
    results = res.results

    mimic = np.concatenate([results[i]["mimic"] for i in range(NCORES)], axis=0)

    # stack per-core stats -> [B, cols], sum the per-chunk partials in f64
    sa = np.concatenate(
        [results[i]["sa"] for i in range(NCORES)], axis=0
    ).astype(np.float64).reshape(B, NCH, NS_ACT).sum(axis=1)
    sv = np.concatenate(
        [results[i]["sv"] for i in range(NCORES)], axis=0
    ).astype(np.float64).reshape(B, NCH, NS_DVE).sum(axis=1)

    s1 = sa[:, 0:4].T      # [4, B]
    s2 = sa[:, 4:8].T
    s3 = sa[:, 8:12].T
    ssq = sa[:, 12:16].T
    A = sv[:, 0:4].T
    Bm = sv[:, 4:8].T
    dots = sv[:, 8:14].T   # [6, B]

    # target logits gathered on host
    bidx = np.arange(B)
    tv = np.stack([o[bidx, tgt].astype(np.float64) for o in outs])  # [4, B]

    CE = np.log(s1) - tv
    E = A - Bm
    KD = (T * T / C) * ((0.2 / T) * E / s3 - (np.log(s3) - np.log(s2)))
    nsq = ssq - tv * tv
    dal = np.zeros(B)
    for idx, (j, k) in enumerate(PAIRS):
        d = dots[idx] - tv[j] * tv[k]
        dal += d / np.sqrt(nsq[j] * nsq[k])
    L = (CE + KD).sum(axis=0) + 0.7 * CE.min(axis=0) + 0.3 * dal
    loss = np.asarray(L.mean(), dtype=np.float32)
    return loss, mimic
